# revision 28
# baseline (speedup 1.0000x reference)
"""Trainium2 Bass kernel for DrBCEncoder-style GNN message passing.

Strategy (8 NeuronCores, SPMD, dst-sharded nodes):
  - Nodes dst-sharded: core c owns rows [c*12500, (c+1)*12500), padded to
    12544 = 98*128 rows (total padded node space 100352).
  - Activations in HBM per layer:
      h_full  [50176, 128] bf16 = 100352 packed 128-byte rows viewed as
              256B row-PAIRS (the dma_gather elem granularity). AllGather
              output, gather source.
      shard   [12544, 64] f32 local shard (residual source, exact).
  - Per 128-dst tile: edges bucketed per (tile, bank, src-parity); the
    2 banks split the pair-index space so indices fit int16 (25088 < 32768).
    Gather elem = 256B = the PAIR containing src; the correct half is
    selected statically per chunk via the matmul stationary slice
    feats[:, k, 64*parity : 64*parity+64] (each bucket is single-parity).
    Edges sorted by src within bucket for HBM row locality. Exact per-call
    counts via count registers (8-deep rotation); pads are trailing idx=-1
    (desc-gen skips them).
  - segment-sum as matmul: PSUM[64f, 128dst] += feats_k[128e, 64f].T @
    oh_k[128e, 128dst] with host-built one-hot (inv_deg folded) per tile.
  - Self+neigh projection fused: stat128 = [hT; nmT] (hT via PE transpose
    of the bf16 shard row tile, nmT copied from the agg PSUM), then one
    matmul z[128n, 64] = stat128.T @ [Ws'; Wn'].
  - LayerNorm in f32 on the free axis, relu + residual (f32 shard), then
    store f32 shard + bf16 packed shard, AllGather for the next layer.

Host-side work is index preprocessing only (edge sort/bucketing, degree
bincount, layout packing, weight transposes, bf16 casts).
"""
import sys

sys.path.insert(0, "/opt/trn_rl_repo")

import ml_dtypes
import numpy as np

import concourse.bass as bass
import concourse.bacc as bacc
import concourse.tile as tile
from concourse import mybir
from concourse.bass_utils import run_bass_kernel_spmd

NCORES = 8
N_NODES = 100000
NODES_PER_CORE = 12500
PAD_PER_CORE = 12544            # 98 * 128
N_PAD = NCORES * PAD_PER_CORE   # 100352
TILES = PAD_PER_CORE // 128     # 98
HALF = PAD_PER_CORE // 2        # 6272 rows per AllGather half
HALF_T = HALF // 128            # 49 tiles per half
PAIRS = N_PAD // 2              # 50176 256B pair-rows
BANKS = 2                       # bank b = all cores' half-b rows
BANK_PAIRS = NCORES * HALF // 2  # 25088 (< 32768 for int16 indices)
BUCKETS = 4                     # (bank << 1) | src_parity
HID = 64
ROW = 2 * HID                   # gather elem: 128 bf16 = 256B = 2 rows
IN_DIM = 8
N_LAYERS = 3
LN_EPS = 1e-5

F32 = mybir.dt.float32
BF16 = mybir.dt.bfloat16
F8 = mybir.dt.float8e4
I16 = mybir.dt.int16
AOT = mybir.AluOpType
ACT_F = mybir.ActivationFunctionType
BF = ml_dtypes.bfloat16
F8NP = mybir.dt.np(mybir.dt.float8e4)

_program_cache = {}

import os
DBG_NQ = int(os.environ.get("GNN_NQ", "4"))        # gather queues used
DBG_HOSTOH = os.environ.get("GNN_HOSTOH", "1") == "1"  # host-precomputed oh
DBG_SORT = os.environ.get("GNN_SORT", "1") == "1"  # sort buckets by src
DBG_SP = os.environ.get("GNN_SP", "0") == "1"      # force single_packet
DBG_SCRATCH = int(os.environ.get("GNN_SCRATCH", "65536"))


def _remap(v):
    return (v // NODES_PER_CORE) * PAD_PER_CORE + (v % NODES_PER_CORE)


def _build_program(l_bank, affine_trivial):
    """SPMD Bass program. l_bank: padded edges per (tile, bucket) slot.
    affine_trivial: gammas==1 and betas==0, skip the two affine ops."""
    cb = l_bank // 128          # chunks per bucket
    C = BUCKETS * cb            # chunks per tile
    lb16 = l_bank // 16

    nc = bacc.Bacc("TRN2", target_bir_lowering=False, debug=False,
                   num_devices=NCORES, num_swdge_queues=DBG_NQ,
                   dynamic_dma_scratch_size=DBG_SCRATCH)

    idx_in = nc.dram_tensor("idx", [TILES, 128, BUCKETS * lb16], I16,
                            kind="ExternalInput")
    meta_in = nc.dram_tensor("meta", [TILES, 128, 2 * C], F32,
                             kind="ExternalInput")
    xt_in = nc.dram_tensor("xt", [IN_DIM, PAD_PER_CORE], F32,
                           kind="ExternalInput")
    w_in_t = nc.dram_tensor("w_in_t", [IN_DIM, HID], F32, kind="ExternalInput")
    w2_in = nc.dram_tensor("w2", [N_LAYERS, 2 * HID, HID], BF16,
                           kind="ExternalInput")
    bias_b = nc.dram_tensor("bias_b", [N_LAYERS, 128, HID], F32,
                            kind="ExternalInput")
    gamma_b = nc.dram_tensor("gamma_b", [N_LAYERS, 128, HID], F32,
                             kind="ExternalInput")
    beta_b = nc.dram_tensor("beta_b", [N_LAYERS, 128, HID], F32,
                            kind="ExternalInput")
    b_in_b = nc.dram_tensor("b_in_b", [128, HID], F32, kind="ExternalInput")
    iota_in = nc.dram_tensor("iota", [128, 128], BF16, kind="ExternalInput")
    ident_in = nc.dram_tensor("ident", [128, 128], BF16, kind="ExternalInput")
    cnt_in = nc.dram_tensor("cnt", [1, TILES * BANKS], mybir.dt.int32,
                            kind="ExternalInput")
    oh_in = None
    if DBG_HOSTOH:
        oh_in = nc.dram_tensor("ohp", [TILES, 128, C * 128], F8,
                               kind="ExternalInput")
    invb_in = nc.dram_tensor("invb", [HID, TILES * 128], BF16,
                             kind="ExternalInput")
    h_out = nc.dram_tensor("h_out", [PAD_PER_CORE, HID], F32,
                           kind="ExternalOutput")

    with tile.TileContext(nc) as tc:
        with (
            tc.tile_pool(name="const", bufs=1) as cp,
            tc.tile_pool(name="io", bufs=6) as iop,
            tc.tile_pool(name="feats", bufs=6) as fp,
            tc.tile_pool(name="oh", bufs=(3 if DBG_HOSTOH else 8)) as ohp,
            tc.tile_pool(name="ln", bufs=3) as lnp,
            tc.tile_pool(name="hb", bufs=3) as hbp,
            tc.tile_pool(name="st", bufs=3) as stp,
            tc.tile_pool(name="ps_agg", bufs=2, space="PSUM") as ps_agg,
            tc.tile_pool(name="ps_tp", bufs=2, space="PSUM") as ps_tp,
            tc.tile_pool(name="ps_z", bufs=2, space="PSUM") as ps_z,
            tc.tile_pool(name="dram", bufs=1, space="DRAM") as dp,
        ):
            # ---- constants ----
            identb_t = cp.tile([128, 128], BF16, tag="identb")
            nc.sync.dma_start(identb_t[:], ident_in[:])
            iota_t = None
            if not DBG_HOSTOH:
                iota_t = cp.tile([128, 128], BF16, tag="iota")
                nc.sync.dma_start(iota_t[:], iota_in[:])
            cnt_sb = cp.tile([1, TILES * BANKS], mybir.dt.int32, tag="cnt")
            nc.sync.dma_start(cnt_sb[:], cnt_in[:])
            eps_t = cp.tile([128, 1], F32, tag="eps")
            nc.vector.memset(eps_t[:], LN_EPS)
            w_in_sb = cp.tile([IN_DIM, HID], F32, tag="w_in")
            nc.sync.dma_start(w_in_sb[:], w_in_t[:])
            b_in_sb = cp.tile([128, HID], F32, tag="b_in")
            nc.sync.dma_start(b_in_sb[:], b_in_b[:])
            invb_sb = cp.tile([HID, TILES * 128], BF16, tag="invb")
            nc.sync.dma_start(invb_sb[:], invb_in[:])
            w2_sb, bias_sb, gamma_sb, beta_sb = [], [], [], []
            for l in range(N_LAYERS):
                w1 = cp.tile([2 * HID, HID], BF16, tag=f"w2_{l}")
                nc.sync.dma_start(w1[:], w2_in[l])
                w2_sb.append(w1)
                b1 = cp.tile([128, HID], F32, tag=f"bias{l}")
                nc.sync.dma_start(b1[:], bias_b[l])
                bias_sb.append(b1)
                if not affine_trivial:
                    g1 = cp.tile([128, HID], F32, tag=f"gamma{l}")
                    nc.sync.dma_start(g1[:], gamma_b[l])
                    gamma_sb.append(g1)
                    be1 = cp.tile([128, HID], F32, tag=f"beta{l}")
                    nc.sync.dma_start(be1[:], beta_b[l])
                    beta_sb.append(be1)

            # ---- DRAM buffers ----
            # gather bank b = AllGather of all cores' half-b shard rows, so
            # the first AllGather can fire mid-layer (after tile HALF_T-1).
            h_bufs = [
                [dp.tile([BANK_PAIRS, ROW], BF16, tag=f"h_buf{i}_{b}",
                         name=f"h_buf{i}_{b}", addr_space="Shared")
                 for b in range(BANKS)]
                for i in range(N_LAYERS)
            ]
            pads = [
                dp.tile([PAD_PER_CORE, HID], BF16, tag=f"pad{i}",
                        name=f"pad{i}")
                for i in range(N_LAYERS)
            ]
            shards = [
                dp.tile([PAD_PER_CORE, HID], F32, tag=f"shard{i}",
                        name=f"shard{i}")
                for i in range(N_LAYERS)
            ]

            # zero the feats pool buffers once: gather skips trailing pad
            # slots (idx=-1) leaving stale bytes that must stay finite.
            for _ in range(6):
                fz = fp.tile([128, C, ROW], BF16, tag="feats")
                nc.vector.memset(fz[:], 0.0)

            # ---- phase 0: h0 = relu(x @ W_in.T + b_in) for own shard ----
            for t in range(TILES):
                xt_sb = iop.tile([IN_DIM, 128], F32, tag="xt")
                nc.sync.dma_start(xt_sb[:], xt_in[:, t * 128:(t + 1) * 128])
                h0_ps = ps_z.tile([128, HID], F32, tag="z")
                nc.tensor.matmul(h0_ps[:], xt_sb[:], w_in_sb[:],
                                 start=True, stop=True)
                h0_sb = lnp.tile([128, HID], F32, tag="hnew")
                nc.vector.scalar_tensor_tensor(
                    h0_sb[:], h0_ps[:], 0.0, b_in_sb[:], AOT.bypass, AOT.add)
                h0r_sb = lnp.tile([128, HID], F32, tag="hnew2")
                nc.scalar.activation(h0r_sb[:], h0_sb[:], ACT_F.Relu)
                hb = hbp.tile([128, HID], BF16, tag="hb")
                nc.scalar.copy(hb[:], h0r_sb[:])
                nc.sync.dma_start(shards[0][t * 128:(t + 1) * 128, :],
                                  h0r_sb[:])
                nc.sync.dma_start(pads[0][t * 128:(t + 1) * 128, :], hb[:])
                if t == HALF_T - 1:
                    nc.gpsimd.collective_compute(
                        "AllGather", AOT.bypass,
                        ins=[pads[0][0:HALF, :].opt()],
                        outs=[h_bufs[0][0].opt()],
                        replica_groups=[list(range(NCORES))])
            nc.gpsimd.collective_compute(
                "AllGather", AOT.bypass,
                ins=[pads[0][HALF:, :].opt()], outs=[h_bufs[0][1].opt()],
                replica_groups=[list(range(NCORES))])

            # ---- layers ----
            # depth-8 register rotation per bucket: the WAR dep between a
            # gather and the count reload for the same register otherwise
            # head-of-line-blocks the Pool sequencer and serializes queues.
            RDEPTH = 8
            cnt_regs = [[nc.gpsimd.alloc_register(f"cnt{b}_{r}")
                         for b in range(BANKS)] for r in range(RDEPTH)]
            for l in range(N_LAYERS):
                last = l == N_LAYERS - 1
                for t in range(TILES):
                    im_t = iop.tile([128, BUCKETS * lb16], I16, tag="idx")
                    nc.sync.dma_start(im_t[:], idx_in[t])
                    if not DBG_HOSTOH:
                        meta_t = iop.tile([128, 2 * C], F32, tag="meta")
                        nc.sync.dma_start(meta_t[:], meta_in[t])
                        meta = meta_t

                    # one merged gather per bank spanning its (even|odd)
                    # parity segments; the count register covers the whole
                    # even segment (middle pads are idx=0, zeroed by oh)
                    # plus the real odd edges (trailing -1 never reached).
                    feats = fp.tile([128, C, ROW], BF16, tag="feats")
                    for b in range(BANKS):
                        g = t * BANKS + b
                        nreg = cnt_regs[t % RDEPTH][b]
                        nc.gpsimd.reg_load(nreg, cnt_sb[0:1, g:g + 1])
                        nc.gpsimd.dma_gather(
                            feats[:, 2 * b * cb:(2 * b + 2) * cb, :],
                            h_bufs[l][b][:],
                            im_t[:, 2 * b * lb16:(2 * b + 2) * lb16],
                            2 * l_bank, nreg, ROW,
                            single_packet=False,
                            queue_num=(2 * (t & 1) + b) % DBG_NQ)

                    agg = ps_agg.tile([HID, 128], F32, tag="agg")
                    if DBG_HOSTOH:
                        oh_t = ohp.tile([128, C * 128], F8, tag="oh")
                        nc.sync.dma_start(oh_t[:], oh_in[t])
                        for k in range(C):
                            par = (k // cb) & 1
                            nc.tensor.matmul(
                                agg[:],
                                feats[:, k, HID * par:HID * par + HID],
                                oh_t[:, k * 128:(k + 1) * 128],
                                start=(k == 0), stop=(k == C - 1))
                    else:
                        for k in range(C):
                            par = (k // cb) & 1
                            oh = ohp.tile([128, 128], BF16, tag="oh")
                            nc.vector.tensor_scalar(
                                oh[:], iota_t[:],
                                meta[:, k:k + 1], None, AOT.is_equal)
                            nc.tensor.matmul(
                                agg[:],
                                feats[:, k, HID * par:HID * par + HID],
                                oh[:],
                                start=(k == 0), stop=(k == C - 1))

                    # stat128 = [hT ; nmT] for the fused z matmul.
                    # nmT = agg * inv_deg[dst]; tensor_tensor is the 1-port
                    # DVE class (no SWDGE port convoy).
                    stat128 = stp.tile([128, 128], BF16, tag="stat")
                    nc.vector.tensor_tensor(
                        stat128[HID:128, :], agg[:],
                        invb_sb[:, t * 128:(t + 1) * 128], AOT.mult)

                    h_t = iop.tile([128, HID], F32, tag="h_t")
                    nc.scalar.dma_start(
                        h_t[:], shards[l][t * 128:(t + 1) * 128, :])
                    # ACT copy, NOT nc.vector: a DVE cast enters 2-port perf
                    # mode and locks GpSimd out of the shared SBUF port,
                    # stalling SWDGE descriptor generation for the gathers.
                    hbt = lnp.tile([128, HID], BF16, tag="hbt")
                    nc.scalar.copy(hbt[:], h_t[:])
                    tp_ps = ps_tp.tile([HID, 128], BF16, tag="tp")
                    nc.tensor.transpose(tp_ps[:], hbt[:], identb_t[:])
                    nc.scalar.copy(stat128[0:HID, :], tp_ps[:])

                    z_ps = ps_z.tile([128, HID], F32, tag="z")
                    nc.tensor.matmul(z_ps[:], stat128[:], w2_sb[l][:],
                                     start=True, stop=True)

                    # LayerNorm + affine + relu + residual
                    stats = lnp.tile([128, 2], F32, tag="stats")
                    zb = lnp.tile([128, HID], F32, tag="zb")
                    nc.vector.scalar_tensor_tensor(
                        zb[:], z_ps[:], 0.0, bias_sb[l][:],
                        AOT.bypass, AOT.add, accum_out=stats[:, 0:1])
                    zsq = lnp.tile([128, HID], F32, tag="zsq")
                    nc.scalar.activation(zsq[:], zb[:], ACT_F.Square,
                                         accum_out=stats[:, 1:2])
                    # tensor_scalar enters DVE 2-port perf mode and convoys
                    # with SWDGE — use scalar_tensor_tensor / tensor_tensor
                    # (1-port) and ACT scale+bias instead.
                    mstat = lnp.tile([128, 2], F32, tag="mstat")
                    nc.vector.scalar_tensor_tensor(
                        mstat[:], stats[:, 0:2], 1.0 / HID, stats[:, 0:2],
                        AOT.mult, AOT.bypass)
                    m2 = lnp.tile([128, 1], F32, tag="m2")
                    nc.vector.tensor_tensor(
                        m2[:], mstat[:, 0:1], mstat[:, 0:1], AOT.mult)
                    var = lnp.tile([128, 1], F32, tag="var")
                    nc.vector.tensor_tensor(
                        var[:], mstat[:, 1:2], m2[:], AOT.subtract)
                    std = lnp.tile([128, 1], F32, tag="std")
                    nc.scalar.activation(std[:], var[:], ACT_F.Sqrt,
                                         bias=eps_t[:])
                    rstd = lnp.tile([128, 1], F32, tag="rstd")
                    nc.vector.reciprocal(rstd[:], std[:])
                    mr = lnp.tile([128, 1], F32, tag="mr")
                    nc.vector.tensor_tensor(
                        mr[:], mstat[:, 0:1], rstd[:], AOT.mult)
                    nmr = lnp.tile([128, 1], F32, tag="nmr")
                    nc.vector.scalar_tensor_tensor(
                        nmr[:], mr[:], -1.0, mr[:], AOT.mult, AOT.bypass)
                    t2 = lnp.tile([128, HID], F32, tag="t2")
                    nc.scalar.activation(t2[:], zb[:], ACT_F.Identity,
                                         bias=nmr[:], scale=rstd[:])
                    t4 = t2
                    if not affine_trivial:
                        t3 = lnp.tile([128, HID], F32, tag="t3")
                        nc.vector.scalar_tensor_tensor(
                            t3[:], t2[:], 0.0, gamma_sb[l][:],
                            AOT.bypass, AOT.mult)
                        t4 = lnp.tile([128, HID], F32, tag="t4")
                        nc.vector.scalar_tensor_tensor(
                            t4[:], t3[:], 0.0, beta_sb[l][:],
                            AOT.bypass, AOT.add)
                    h_new = lnp.tile([128, HID], F32, tag="hnew")
                    nc.vector.scalar_tensor_tensor(
                        h_new[:], t4[:], 0.0, h_t[:], AOT.max, AOT.add)

                    if last:
                        nc.sync.dma_start(
                            h_out[t * 128:(t + 1) * 128, :], h_new[:])
                    else:
                        hb = hbp.tile([128, HID], BF16, tag="hb")
                        nc.scalar.copy(hb[:], h_new[:])
                        nc.sync.dma_start(
                            shards[l + 1][t * 128:(t + 1) * 128, :], h_new[:])
                        nc.sync.dma_start(
                            pads[l + 1][t * 128:(t + 1) * 128, :], hb[:])
                        if t == HALF_T - 1:
                            nc.gpsimd.collective_compute(
                                "AllGather", AOT.bypass,
                                ins=[pads[l + 1][0:HALF, :].opt()],
                                outs=[h_bufs[l + 1][0].opt()],
                                replica_groups=[list(range(NCORES))])
                if not last:
                    nc.gpsimd.collective_compute(
                        "AllGather", AOT.bypass,
                        ins=[pads[l + 1][HALF:, :].opt()],
                        outs=[h_bufs[l + 1][1].opt()],
                        replica_groups=[list(range(NCORES))])

    nc.compile()
    return nc


def _preprocess(x, edge_src, edge_dst, W_in, b_in, Ws_self, Ws_neigh,
                biases, gammas, betas):
    """Pure index/layout preprocessing on the host."""
    src = edge_src.astype(np.int64)
    dst = edge_dst.astype(np.int64)
    rsrc = _remap(src)
    rdst = _remap(dst)

    tile_g = rdst // 128              # global tile id in padded space, 0..783
    dst_loc = (rdst % 128).astype(np.int64)
    # bank b holds the AllGather of all cores' half-b rows:
    # h_bufs[l][b][c*HALF/2 + jb/2] = 256B pair of core c rows (jb, jb+1)
    src_core = rsrc // PAD_PER_CORE
    src_j = rsrc % PAD_PER_CORE
    bank = (src_j >= HALF).astype(np.int64)
    jb = src_j - bank * HALF
    parity = (rsrc & 1).astype(np.int64)
    idx_loc = (src_core * (HALF // 2) + (jb >> 1)).astype(np.int16)

    deg = np.bincount(dst, minlength=N_NODES)
    invdeg = np.where(deg > 0, 1.0 / np.maximum(deg, 1), 0.0).astype(np.float32)
    inv_e = invdeg[dst]

    n_groups = NCORES * TILES * BUCKETS
    bucket = (bank << 1) | parity
    key = tile_g * BUCKETS + bucket
    if DBG_SORT:
        order = np.lexsort((rsrc, key))
    else:
        order = np.argsort(key, kind="stable")
    key_s = key[order]
    counts = np.bincount(key_s, minlength=n_groups)
    l_bank = max(256, int(np.ceil(counts.max() / 128)) * 128)
    cb = l_bank // 128
    C = BUCKETS * cb
    lb16 = l_bank // 16

    starts = np.zeros(n_groups, dtype=np.int64)
    starts[1:] = np.cumsum(counts)[:-1]
    rank = np.arange(len(src)) - starts[key_s]
    pos = key_s * l_bank + rank       # global padded position

    total = n_groups * l_bank
    idx_full = np.full(total, -1, dtype=np.int16)  # pad: skipped by ucode
    idx_full[pos] = idx_loc[order]
    # even-parity buckets sit first within a merged bank call: their pad
    # slots are mid-stream, must be valid reads (idx 0, oh column is 0)
    counts_g = counts.reshape(-1, 2)      # [(tile,bank), parity]
    for gidx in range(0, n_groups, 2):
        c_even = counts[gidx]
        if c_even < l_bank:
            idx_full[gidx * l_bank + c_even:(gidx + 1) * l_bank] = 0
    dstl_full = np.full(total, -1.0, dtype=np.float32)
    dstl_full[pos] = dst_loc[order].astype(np.float32)
    inv_full = np.zeros(total, dtype=np.float32)
    inv_full[pos] = inv_e[order]

    # idx: [784, BUCKETS, l_bank] -> wrap16 -> replicate to 128 partitions
    idx_w = idx_full.reshape(NCORES * TILES, BUCKETS, lb16, 16)
    idx_w = idx_w.transpose(0, 1, 3, 2)                # [784, U, 16, lb16]
    idx_w = np.broadcast_to(idx_w[:, :, None, :, :],
                            (NCORES * TILES, BUCKETS, 8, 16, lb16))
    idx_w = idx_w.transpose(0, 2, 3, 1, 4).reshape(
        NCORES, TILES, 128, BUCKETS * lb16)

    # meta: positions within a tile wrap mod 128 across all chunks
    dstl_w = dstl_full.reshape(NCORES * TILES, C, 128).transpose(0, 2, 1)
    inv_w = inv_full.reshape(NCORES * TILES, C, 128).transpose(0, 2, 1)
    meta = np.concatenate([dstl_w, inv_w], axis=2).reshape(
        NCORES, TILES, 128, 2 * C).astype(np.float32)
    idx_w = np.ascontiguousarray(idx_w)
    meta = np.ascontiguousarray(meta)

    oh_host = None
    if DBG_HOSTOH:
        # 0/1 one-hot tiles in fp8 (exact), edge-major partitions; inv_deg
        # is applied on-device per dst column (invb) instead of per edge.
        bucket_e = key_s % BUCKETS
        rank_e = rank  # within (tile, bucket) group, aligned with `order`
        tile_e = key_s // BUCKETS
        k_e = bucket_e * cb + rank_e // 128      # chunk within tile
        e_loc = rank_e % 128                      # partition within chunk
        flat = ((tile_e * 128 + e_loc) * C + k_e) * 128 + dst_loc[order]
        oh_host = np.zeros(NCORES * TILES * 128 * C * 128, dtype=np.uint8)
        one_f8 = np.ones((), dtype=F8NP).view(np.uint8)
        oh_host[flat] = one_f8
        oh_host = oh_host.view(F8NP).reshape(NCORES, TILES, 128, C * 128)

    # per-dst inv_deg, broadcast to 64 partitions: [NCORES, 64, 12544]
    invp = np.zeros(N_PAD, dtype=np.float32)
    invp[_remap(np.arange(N_NODES))] = invdeg
    invb = np.ascontiguousarray(np.broadcast_to(
        invp.reshape(NCORES, 1, PAD_PER_CORE),
        (NCORES, HID, PAD_PER_CORE)).astype(BF))

    # xT per core
    xp = np.zeros((N_PAD, IN_DIM), dtype=np.float32)
    xp[_remap(np.arange(N_NODES))] = x
    xp = xp.reshape(NCORES, PAD_PER_CORE, IN_DIM)

    w_in_t = np.ascontiguousarray(W_in.T.astype(np.float32))
    ws_t = Ws_self.transpose(0, 2, 1).astype(np.float32)
    wn_t = Ws_neigh.transpose(0, 2, 1).astype(np.float32)
    w2 = np.ascontiguousarray(
        np.concatenate([ws_t, wn_t], axis=1)).astype(BF)  # [L, 128, 64]
    bias_b = np.ascontiguousarray(
        np.broadcast_to(biases[:, None, :],
                        (N_LAYERS, 128, HID)).astype(np.float32))
    gamma_b = np.ascontiguousarray(
        np.broadcast_to(gammas[:, None, :],
                        (N_LAYERS, 128, HID)).astype(np.float32))
    beta_b = np.ascontiguousarray(
        np.broadcast_to(betas[:, None, :],
                        (N_LAYERS, 128, HID)).astype(np.float32))
    b_in_bc = np.ascontiguousarray(
        np.broadcast_to(b_in[None, :], (128, HID)).astype(np.float32))
    iota = np.tile(np.arange(128, dtype=np.float32), (128, 1)).astype(BF)
    ident = np.eye(128, dtype=np.float32).astype(BF)

    affine_trivial = bool(np.all(gammas == 1.0) and np.all(betas == 0.0))

    # merged-call count per (tile, bank): full even segment + real odds
    counts_pc = (l_bank + counts_g[:, 1]).reshape(
        NCORES, TILES, BANKS).astype(np.int32)

    in_maps = []
    for c in range(NCORES):
        in_maps.append({
            "idx": idx_w[c],
            "meta": meta[c],
            "xt": np.ascontiguousarray(xp[c].T),
            "w_in_t": w_in_t,
            "w2": w2,
            "bias_b": bias_b,
            "gamma_b": gamma_b,
            "beta_b": beta_b,
            "b_in_b": b_in_bc,
            "iota": iota,
            "ident": ident,
            "cnt": np.ascontiguousarray(
                counts_pc[c].reshape(1, TILES * BANKS)),
            "invb": invb[c],
            **({"ohp": oh_host[c]} if DBG_HOSTOH else {}),
        })
    return in_maps, l_bank, affine_trivial


def kernel(**inputs):
    in_maps, l_bank, affine_trivial = _preprocess(
        np.asarray(inputs["x"]), np.asarray(inputs["edge_src"]),
        np.asarray(inputs["edge_dst"]), np.asarray(inputs["W_in"]),
        np.asarray(inputs["b_in"]), np.asarray(inputs["Ws_self"]),
        np.asarray(inputs["Ws_neigh"]), np.asarray(inputs["biases"]),
        np.asarray(inputs["gammas"]), np.asarray(inputs["betas"]))

    key = (l_bank, affine_trivial, DBG_NQ, DBG_HOSTOH, DBG_SP, DBG_SCRATCH)
    if key not in _program_cache:
        _program_cache[key] = _build_program(l_bank, affine_trivial)
    nc = _program_cache[key]

    res = run_bass_kernel_spmd(nc, in_maps, list(range(NCORES)))
    out = np.concatenate(
        [res.results[c]["h_out"][:NODES_PER_CORE] for c in range(NCORES)],
        axis=0)
    return out.astype(np.float32)


# revision 30
# speedup vs baseline: 1.2203x; 1.2203x over previous
"""Trainium2 Bass kernel for DrBCEncoder-style GNN message passing.

Strategy (8 NeuronCores, SPMD, dst-sharded nodes):
  - Nodes dst-sharded: core c owns rows [c*12500, (c+1)*12500), padded to
    12544 = 98*128 rows (total padded node space 100352).
  - Activations in HBM per layer:
      h_full  [50176, 128] bf16 = 100352 packed 128-byte rows viewed as
              256B row-PAIRS (the dma_gather elem granularity). AllGather
              output, gather source.
      shard   [12544, 64] f32 local shard (residual source, exact).
  - Per 128-dst tile: edges bucketed per (tile, bank, src-parity); the
    2 banks split the pair-index space so indices fit int16 (25088 < 32768).
    Gather elem = 256B = the PAIR containing src; the correct half is
    selected statically per chunk via the matmul stationary slice
    feats[:, k, 64*parity : 64*parity+64] (each bucket is single-parity).
    Edges sorted by src within bucket for HBM row locality. Exact per-call
    counts via count registers (8-deep rotation); pads are trailing idx=-1
    (desc-gen skips them).
  - segment-sum as matmul: PSUM[64f, 128dst] += feats_k[128e, 64f].T @
    oh_k[128e, 128dst] with host-built one-hot (inv_deg folded) per tile.
  - Self+neigh projection fused: stat128 = [hT; nmT] (hT via PE transpose
    of the bf16 shard row tile, nmT copied from the agg PSUM), then one
    matmul z[128n, 64] = stat128.T @ [Ws'; Wn'].
  - LayerNorm in f32 on the free axis, relu + residual (f32 shard), then
    store f32 shard + bf16 packed shard, AllGather for the next layer.

Host-side work is index preprocessing only (edge sort/bucketing, degree
bincount, layout packing, weight transposes, bf16 casts).
"""
import sys

sys.path.insert(0, "/opt/trn_rl_repo")

import ml_dtypes
import numpy as np

import concourse.bass as bass
import concourse.bacc as bacc
import concourse.tile as tile
from concourse import mybir
from concourse.bass_utils import run_bass_kernel_spmd

NCORES = 8
N_NODES = 100000
NODES_PER_CORE = 12500
PAD_PER_CORE = 12544            # 98 * 128
N_PAD = NCORES * PAD_PER_CORE   # 100352
TILES = PAD_PER_CORE // 128     # 98
HALF = PAD_PER_CORE // 2        # 6272 rows per AllGather half
HALF_T = HALF // 128            # 49 tiles per half
PAIRS = N_PAD // 2              # 50176 256B pair-rows
BANKS = 2                       # bank b = all cores' half-b rows
BANK_PAIRS = NCORES * HALF // 2  # 25088 (< 32768 for int16 indices)
BUCKETS = 4                     # (bank << 1) | src_parity
HID = 64
ROW = 2 * HID                   # gather elem: 128 bf16 = 256B = 2 rows
IN_DIM = 8
N_LAYERS = 3
LN_EPS = 1e-5

F32 = mybir.dt.float32
BF16 = mybir.dt.bfloat16
F8 = mybir.dt.float8e4
I16 = mybir.dt.int16
AOT = mybir.AluOpType
ACT_F = mybir.ActivationFunctionType
BF = ml_dtypes.bfloat16
F8NP = mybir.dt.np(mybir.dt.float8e4)

_program_cache = {}

import os
DBG_NQ = int(os.environ.get("GNN_NQ", "4"))        # gather queues used
DBG_HOSTOH = os.environ.get("GNN_HOSTOH", "1") == "1"  # host-precomputed oh
DBG_SORT = os.environ.get("GNN_SORT", "1") == "1"  # sort buckets by src
DBG_SP = os.environ.get("GNN_SP", "0") == "1"      # force single_packet
DBG_SCRATCH = int(os.environ.get("GNN_SCRATCH", "65536"))


def _remap(v):
    return (v // NODES_PER_CORE) * PAD_PER_CORE + (v % NODES_PER_CORE)


def _build_program(l_bank, affine_trivial):
    """SPMD Bass program. l_bank: padded edges per (tile, bucket) slot.
    affine_trivial: gammas==1 and betas==0, skip the two affine ops."""
    cb = l_bank // 128          # chunks per bucket
    C = BUCKETS * cb            # chunks per tile
    lb16 = l_bank // 16

    nc = bacc.Bacc("TRN2", target_bir_lowering=False, debug=False,
                   num_devices=NCORES, num_swdge_queues=DBG_NQ,
                   dynamic_dma_scratch_size=DBG_SCRATCH)

    idx_in = nc.dram_tensor("idx", [TILES, 128, BUCKETS * lb16], I16,
                            kind="ExternalInput")
    meta_in = nc.dram_tensor("meta", [TILES, 128, 2 * C], F32,
                             kind="ExternalInput")
    xt_in = nc.dram_tensor("xt", [IN_DIM, PAD_PER_CORE], F32,
                           kind="ExternalInput")
    w_in_t = nc.dram_tensor("w_in_t", [IN_DIM, HID], F32, kind="ExternalInput")
    w2_in = nc.dram_tensor("w2", [N_LAYERS, 2 * HID, HID], BF16,
                           kind="ExternalInput")
    bias_b = nc.dram_tensor("bias_b", [N_LAYERS, 128, HID], F32,
                            kind="ExternalInput")
    gamma_b = nc.dram_tensor("gamma_b", [N_LAYERS, 128, HID], F32,
                             kind="ExternalInput")
    beta_b = nc.dram_tensor("beta_b", [N_LAYERS, 128, HID], F32,
                            kind="ExternalInput")
    b_in_b = nc.dram_tensor("b_in_b", [128, HID], F32, kind="ExternalInput")
    iota_in = nc.dram_tensor("iota", [128, 128], BF16, kind="ExternalInput")
    ident_in = nc.dram_tensor("ident", [128, 128], BF16, kind="ExternalInput")
    cnt_in = nc.dram_tensor("cnt", [1, TILES * BUCKETS], mybir.dt.int32,
                            kind="ExternalInput")
    oh_in = None
    if DBG_HOSTOH:
        oh_in = nc.dram_tensor("ohp", [TILES, 128, C * 128], F8,
                               kind="ExternalInput")
    invb_in = nc.dram_tensor("invb", [HID, TILES * 128], BF16,
                             kind="ExternalInput")
    h_out = nc.dram_tensor("h_out", [PAD_PER_CORE, HID], F32,
                           kind="ExternalOutput")

    with tile.TileContext(nc) as tc:
        with (
            tc.tile_pool(name="const", bufs=1) as cp,
            tc.tile_pool(name="io", bufs=6) as iop,
            tc.tile_pool(name="feats", bufs=6) as fp,
            tc.tile_pool(name="oh", bufs=(3 if DBG_HOSTOH else 8)) as ohp,
            tc.tile_pool(name="ln", bufs=3) as lnp,
            tc.tile_pool(name="hb", bufs=3) as hbp,
            tc.tile_pool(name="st", bufs=3) as stp,
            tc.tile_pool(name="ps_agg", bufs=2, space="PSUM") as ps_agg,
            tc.tile_pool(name="ps_tp", bufs=2, space="PSUM") as ps_tp,
            tc.tile_pool(name="ps_z", bufs=2, space="PSUM") as ps_z,
            tc.tile_pool(name="dram", bufs=1, space="DRAM") as dp,
        ):
            # ---- constants ----
            identb_t = cp.tile([128, 128], BF16, tag="identb")
            nc.sync.dma_start(identb_t[:], ident_in[:])
            iota_t = None
            if not DBG_HOSTOH:
                iota_t = cp.tile([128, 128], BF16, tag="iota")
                nc.sync.dma_start(iota_t[:], iota_in[:])
            cnt_sb = cp.tile([1, TILES * BUCKETS], mybir.dt.int32, tag="cnt")
            nc.sync.dma_start(cnt_sb[:], cnt_in[:])
            eps_t = cp.tile([128, 1], F32, tag="eps")
            nc.vector.memset(eps_t[:], LN_EPS)
            w_in_sb = cp.tile([IN_DIM, HID], F32, tag="w_in")
            nc.sync.dma_start(w_in_sb[:], w_in_t[:])
            b_in_sb = cp.tile([128, HID], F32, tag="b_in")
            nc.sync.dma_start(b_in_sb[:], b_in_b[:])
            invb_sb = cp.tile([HID, TILES * 128], BF16, tag="invb")
            nc.sync.dma_start(invb_sb[:], invb_in[:])
            w2_sb, bias_sb, gamma_sb, beta_sb = [], [], [], []
            for l in range(N_LAYERS):
                w1 = cp.tile([2 * HID, HID], BF16, tag=f"w2_{l}")
                nc.sync.dma_start(w1[:], w2_in[l])
                w2_sb.append(w1)
                b1 = cp.tile([128, HID], F32, tag=f"bias{l}")
                nc.sync.dma_start(b1[:], bias_b[l])
                bias_sb.append(b1)
                if not affine_trivial:
                    g1 = cp.tile([128, HID], F32, tag=f"gamma{l}")
                    nc.sync.dma_start(g1[:], gamma_b[l])
                    gamma_sb.append(g1)
                    be1 = cp.tile([128, HID], F32, tag=f"beta{l}")
                    nc.sync.dma_start(be1[:], beta_b[l])
                    beta_sb.append(be1)

            # ---- DRAM buffers ----
            # gather bank b = AllGather of all cores' half-b shard rows, so
            # the first AllGather can fire mid-layer (after tile HALF_T-1).
            h_bufs = [
                [dp.tile([BANK_PAIRS, ROW], BF16, tag=f"h_buf{i}_{b}",
                         name=f"h_buf{i}_{b}", addr_space="Shared")
                 for b in range(BANKS)]
                for i in range(N_LAYERS)
            ]
            pads = [
                dp.tile([PAD_PER_CORE, HID], BF16, tag=f"pad{i}",
                        name=f"pad{i}")
                for i in range(N_LAYERS)
            ]
            shards = [
                dp.tile([PAD_PER_CORE, HID], F32, tag=f"shard{i}",
                        name=f"shard{i}")
                for i in range(N_LAYERS)
            ]

            # zero the feats pool buffers once: gather skips trailing pad
            # slots (idx=-1) leaving stale bytes that must stay finite.
            for _ in range(6):
                fz = fp.tile([128, C, ROW], BF16, tag="feats")
                nc.vector.memset(fz[:], 0.0)

            # ---- phase 0: h0 = relu(x @ W_in.T + b_in) for own shard ----
            for t in range(TILES):
                xt_sb = iop.tile([IN_DIM, 128], F32, tag="xt")
                nc.sync.dma_start(xt_sb[:], xt_in[:, t * 128:(t + 1) * 128])
                h0_ps = ps_z.tile([128, HID], F32, tag="z")
                nc.tensor.matmul(h0_ps[:], xt_sb[:], w_in_sb[:],
                                 start=True, stop=True)
                h0_sb = lnp.tile([128, HID], F32, tag="hnew")
                nc.vector.scalar_tensor_tensor(
                    h0_sb[:], h0_ps[:], 0.0, b_in_sb[:], AOT.bypass, AOT.add)
                h0r_sb = lnp.tile([128, HID], F32, tag="hnew2")
                nc.scalar.activation(h0r_sb[:], h0_sb[:], ACT_F.Relu)
                hb = hbp.tile([128, HID], BF16, tag="hb")
                nc.scalar.copy(hb[:], h0r_sb[:])
                nc.sync.dma_start(shards[0][t * 128:(t + 1) * 128, :],
                                  h0r_sb[:])
                nc.sync.dma_start(pads[0][t * 128:(t + 1) * 128, :], hb[:])
                if t == HALF_T - 1:
                    nc.gpsimd.collective_compute(
                        "AllGather", AOT.bypass,
                        ins=[pads[0][0:HALF, :].opt()],
                        outs=[h_bufs[0][0].opt()],
                        replica_groups=[list(range(NCORES))])
            nc.gpsimd.collective_compute(
                "AllGather", AOT.bypass,
                ins=[pads[0][HALF:, :].opt()], outs=[h_bufs[0][1].opt()],
                replica_groups=[list(range(NCORES))])

            # ---- layers ----
            # depth-8 register rotation per bucket: the WAR dep between a
            # gather and the count reload for the same register otherwise
            # head-of-line-blocks the Pool sequencer and serializes queues.
            RDEPTH = 8
            cnt_regs = [[nc.gpsimd.alloc_register(f"cnt{b}_{r}")
                         for b in range(BUCKETS)] for r in range(RDEPTH)]
            for l in range(N_LAYERS):
                last = l == N_LAYERS - 1
                for t in range(TILES):
                    im_t = iop.tile([128, BUCKETS * lb16], I16, tag="idx")
                    nc.sync.dma_start(im_t[:], idx_in[t])
                    if not DBG_HOSTOH:
                        meta_t = iop.tile([128, 2 * C], F32, tag="meta")
                        nc.sync.dma_start(meta_t[:], meta_in[t])
                        meta = meta_t

                    feats = fp.tile([128, C, ROW], BF16, tag="feats")
                    for u in range(BUCKETS):
                        bank = u >> 1
                        g = t * BUCKETS + u
                        nreg = cnt_regs[t % RDEPTH][u]
                        nc.gpsimd.reg_load(nreg, cnt_sb[0:1, g:g + 1])
                        nc.gpsimd.dma_gather(
                            feats[:, u * cb:(u + 1) * cb, :],
                            h_bufs[l][bank][:],
                            im_t[:, u * lb16:(u + 1) * lb16],
                            l_bank, nreg, ROW,
                            single_packet=False,
                            queue_num=u % DBG_NQ)

                    agg = ps_agg.tile([HID, 128], F32, tag="agg")
                    if DBG_HOSTOH:
                        oh_t = ohp.tile([128, C * 128], F8, tag="oh")
                        nc.sync.dma_start(oh_t[:], oh_in[t])
                        for k in range(C):
                            par = (k // cb) & 1
                            nc.tensor.matmul(
                                agg[:],
                                feats[:, k, HID * par:HID * par + HID],
                                oh_t[:, k * 128:(k + 1) * 128],
                                start=(k == 0), stop=(k == C - 1))
                    else:
                        for k in range(C):
                            par = (k // cb) & 1
                            oh = ohp.tile([128, 128], BF16, tag="oh")
                            nc.vector.tensor_scalar(
                                oh[:], iota_t[:],
                                meta[:, k:k + 1], None, AOT.is_equal)
                            nc.tensor.matmul(
                                agg[:],
                                feats[:, k, HID * par:HID * par + HID],
                                oh[:],
                                start=(k == 0), stop=(k == C - 1))

                    # stat128 = [hT ; nmT] for the fused z matmul.
                    # nmT = agg * inv_deg[dst]; tensor_tensor is the 1-port
                    # DVE class (no SWDGE port convoy).
                    stat128 = stp.tile([128, 128], BF16, tag="stat")
                    nc.vector.tensor_tensor(
                        stat128[HID:128, :], agg[:],
                        invb_sb[:, t * 128:(t + 1) * 128], AOT.mult)

                    h_t = iop.tile([128, HID], F32, tag="h_t")
                    nc.scalar.dma_start(
                        h_t[:], shards[l][t * 128:(t + 1) * 128, :])
                    # ACT copy, NOT nc.vector: a DVE cast enters 2-port perf
                    # mode and locks GpSimd out of the shared SBUF port,
                    # stalling SWDGE descriptor generation for the gathers.
                    hbt = lnp.tile([128, HID], BF16, tag="hbt")
                    nc.scalar.copy(hbt[:], h_t[:])
                    tp_ps = ps_tp.tile([HID, 128], BF16, tag="tp")
                    nc.tensor.transpose(tp_ps[:], hbt[:], identb_t[:])
                    nc.scalar.copy(stat128[0:HID, :], tp_ps[:])

                    z_ps = ps_z.tile([128, HID], F32, tag="z")
                    nc.tensor.matmul(z_ps[:], stat128[:], w2_sb[l][:],
                                     start=True, stop=True)

                    # LayerNorm + affine + relu + residual
                    stats = lnp.tile([128, 2], F32, tag="stats")
                    zb = lnp.tile([128, HID], F32, tag="zb")
                    nc.vector.scalar_tensor_tensor(
                        zb[:], z_ps[:], 0.0, bias_sb[l][:],
                        AOT.bypass, AOT.add, accum_out=stats[:, 0:1])
                    zsq = lnp.tile([128, HID], F32, tag="zsq")
                    nc.scalar.activation(zsq[:], zb[:], ACT_F.Square,
                                         accum_out=stats[:, 1:2])
                    # tensor_scalar enters DVE 2-port perf mode and convoys
                    # with SWDGE — use scalar_tensor_tensor / tensor_tensor
                    # (1-port) and ACT scale+bias instead.
                    mstat = lnp.tile([128, 2], F32, tag="mstat")
                    nc.vector.scalar_tensor_tensor(
                        mstat[:], stats[:, 0:2], 1.0 / HID, stats[:, 0:2],
                        AOT.mult, AOT.bypass)
                    m2 = lnp.tile([128, 1], F32, tag="m2")
                    nc.vector.tensor_tensor(
                        m2[:], mstat[:, 0:1], mstat[:, 0:1], AOT.mult)
                    var = lnp.tile([128, 1], F32, tag="var")
                    nc.vector.tensor_tensor(
                        var[:], mstat[:, 1:2], m2[:], AOT.subtract)
                    std = lnp.tile([128, 1], F32, tag="std")
                    nc.scalar.activation(std[:], var[:], ACT_F.Sqrt,
                                         bias=eps_t[:])
                    rstd = lnp.tile([128, 1], F32, tag="rstd")
                    nc.vector.reciprocal(rstd[:], std[:])
                    mr = lnp.tile([128, 1], F32, tag="mr")
                    nc.vector.tensor_tensor(
                        mr[:], mstat[:, 0:1], rstd[:], AOT.mult)
                    nmr = lnp.tile([128, 1], F32, tag="nmr")
                    nc.vector.scalar_tensor_tensor(
                        nmr[:], mr[:], -1.0, mr[:], AOT.mult, AOT.bypass)
                    t2 = lnp.tile([128, HID], F32, tag="t2")
                    nc.scalar.activation(t2[:], zb[:], ACT_F.Identity,
                                         bias=nmr[:], scale=rstd[:])
                    t4 = t2
                    if not affine_trivial:
                        t3 = lnp.tile([128, HID], F32, tag="t3")
                        nc.vector.scalar_tensor_tensor(
                            t3[:], t2[:], 0.0, gamma_sb[l][:],
                            AOT.bypass, AOT.mult)
                        t4 = lnp.tile([128, HID], F32, tag="t4")
                        nc.vector.scalar_tensor_tensor(
                            t4[:], t3[:], 0.0, beta_sb[l][:],
                            AOT.bypass, AOT.add)
                    h_new = lnp.tile([128, HID], F32, tag="hnew")
                    nc.vector.scalar_tensor_tensor(
                        h_new[:], t4[:], 0.0, h_t[:], AOT.max, AOT.add)

                    if last:
                        nc.sync.dma_start(
                            h_out[t * 128:(t + 1) * 128, :], h_new[:])
                    else:
                        hb = hbp.tile([128, HID], BF16, tag="hb")
                        nc.scalar.copy(hb[:], h_new[:])
                        nc.sync.dma_start(
                            shards[l + 1][t * 128:(t + 1) * 128, :], h_new[:])
                        nc.sync.dma_start(
                            pads[l + 1][t * 128:(t + 1) * 128, :], hb[:])
                        if t == HALF_T - 1:
                            nc.gpsimd.collective_compute(
                                "AllGather", AOT.bypass,
                                ins=[pads[l + 1][0:HALF, :].opt()],
                                outs=[h_bufs[l + 1][0].opt()],
                                replica_groups=[list(range(NCORES))])
                if not last:
                    nc.gpsimd.collective_compute(
                        "AllGather", AOT.bypass,
                        ins=[pads[l + 1][HALF:, :].opt()],
                        outs=[h_bufs[l + 1][1].opt()],
                        replica_groups=[list(range(NCORES))])

    nc.compile()
    return nc


def _preprocess(x, edge_src, edge_dst, W_in, b_in, Ws_self, Ws_neigh,
                biases, gammas, betas):
    """Pure index/layout preprocessing on the host."""
    src = edge_src.astype(np.int64)
    dst = edge_dst.astype(np.int64)
    rsrc = _remap(src)
    rdst = _remap(dst)

    tile_g = rdst // 128              # global tile id in padded space, 0..783
    dst_loc = (rdst % 128).astype(np.int64)
    # bank b holds the AllGather of all cores' half-b rows:
    # h_bufs[l][b][c*HALF/2 + jb/2] = 256B pair of core c rows (jb, jb+1)
    src_core = rsrc // PAD_PER_CORE
    src_j = rsrc % PAD_PER_CORE
    bank = (src_j >= HALF).astype(np.int64)
    jb = src_j - bank * HALF
    parity = (rsrc & 1).astype(np.int64)
    idx_loc = (src_core * (HALF // 2) + (jb >> 1)).astype(np.int16)

    deg = np.bincount(dst, minlength=N_NODES)
    invdeg = np.where(deg > 0, 1.0 / np.maximum(deg, 1), 0.0).astype(np.float32)
    inv_e = invdeg[dst]

    n_groups = NCORES * TILES * BUCKETS
    bucket = (bank << 1) | parity
    key = tile_g * BUCKETS + bucket
    if DBG_SORT:
        order = np.lexsort((rsrc, key))
    else:
        order = np.argsort(key, kind="stable")
    key_s = key[order]
    counts = np.bincount(key_s, minlength=n_groups)
    l_bank = max(256, int(np.ceil(counts.max() / 128)) * 128)
    cb = l_bank // 128
    C = BUCKETS * cb
    lb16 = l_bank // 16

    starts = np.zeros(n_groups, dtype=np.int64)
    starts[1:] = np.cumsum(counts)[:-1]
    rank = np.arange(len(src)) - starts[key_s]
    pos = key_s * l_bank + rank       # global padded position

    total = n_groups * l_bank
    idx_full = np.full(total, -1, dtype=np.int16)  # pad: skipped by ucode
    idx_full[pos] = idx_loc[order]
    dstl_full = np.full(total, -1.0, dtype=np.float32)
    dstl_full[pos] = dst_loc[order].astype(np.float32)
    inv_full = np.zeros(total, dtype=np.float32)
    inv_full[pos] = inv_e[order]

    # idx: [784, BUCKETS, l_bank] -> wrap16 -> replicate to 128 partitions
    idx_w = idx_full.reshape(NCORES * TILES, BUCKETS, lb16, 16)
    idx_w = idx_w.transpose(0, 1, 3, 2)                # [784, U, 16, lb16]
    idx_w = np.broadcast_to(idx_w[:, :, None, :, :],
                            (NCORES * TILES, BUCKETS, 8, 16, lb16))
    idx_w = idx_w.transpose(0, 2, 3, 1, 4).reshape(
        NCORES, TILES, 128, BUCKETS * lb16)

    # meta: positions within a tile wrap mod 128 across all chunks
    dstl_w = dstl_full.reshape(NCORES * TILES, C, 128).transpose(0, 2, 1)
    inv_w = inv_full.reshape(NCORES * TILES, C, 128).transpose(0, 2, 1)
    meta = np.concatenate([dstl_w, inv_w], axis=2).reshape(
        NCORES, TILES, 128, 2 * C).astype(np.float32)
    idx_w = np.ascontiguousarray(idx_w)
    meta = np.ascontiguousarray(meta)

    oh_host = None
    if DBG_HOSTOH:
        # 0/1 one-hot tiles in fp8 (exact), edge-major partitions; inv_deg
        # is applied on-device per dst column (invb) instead of per edge.
        bucket_e = key_s % BUCKETS
        rank_e = rank  # within (tile, bucket) group, aligned with `order`
        tile_e = key_s // BUCKETS
        k_e = bucket_e * cb + rank_e // 128      # chunk within tile
        e_loc = rank_e % 128                      # partition within chunk
        flat = ((tile_e * 128 + e_loc) * C + k_e) * 128 + dst_loc[order]
        oh_host = np.zeros(NCORES * TILES * 128 * C * 128, dtype=np.uint8)
        one_f8 = np.ones((), dtype=F8NP).view(np.uint8)
        oh_host[flat] = one_f8
        oh_host = oh_host.view(F8NP).reshape(NCORES, TILES, 128, C * 128)

    # per-dst inv_deg, broadcast to 64 partitions: [NCORES, 64, 12544]
    invp = np.zeros(N_PAD, dtype=np.float32)
    invp[_remap(np.arange(N_NODES))] = invdeg
    invb = np.ascontiguousarray(np.broadcast_to(
        invp.reshape(NCORES, 1, PAD_PER_CORE),
        (NCORES, HID, PAD_PER_CORE)).astype(BF))

    # xT per core
    xp = np.zeros((N_PAD, IN_DIM), dtype=np.float32)
    xp[_remap(np.arange(N_NODES))] = x
    xp = xp.reshape(NCORES, PAD_PER_CORE, IN_DIM)

    w_in_t = np.ascontiguousarray(W_in.T.astype(np.float32))
    ws_t = Ws_self.transpose(0, 2, 1).astype(np.float32)
    wn_t = Ws_neigh.transpose(0, 2, 1).astype(np.float32)
    w2 = np.ascontiguousarray(
        np.concatenate([ws_t, wn_t], axis=1)).astype(BF)  # [L, 128, 64]
    bias_b = np.ascontiguousarray(
        np.broadcast_to(biases[:, None, :],
                        (N_LAYERS, 128, HID)).astype(np.float32))
    gamma_b = np.ascontiguousarray(
        np.broadcast_to(gammas[:, None, :],
                        (N_LAYERS, 128, HID)).astype(np.float32))
    beta_b = np.ascontiguousarray(
        np.broadcast_to(betas[:, None, :],
                        (N_LAYERS, 128, HID)).astype(np.float32))
    b_in_bc = np.ascontiguousarray(
        np.broadcast_to(b_in[None, :], (128, HID)).astype(np.float32))
    iota = np.tile(np.arange(128, dtype=np.float32), (128, 1)).astype(BF)
    ident = np.eye(128, dtype=np.float32).astype(BF)

    affine_trivial = bool(np.all(gammas == 1.0) and np.all(betas == 0.0))

    counts_pc = counts.reshape(NCORES, TILES, BUCKETS).astype(np.int32)

    in_maps = []
    for c in range(NCORES):
        in_maps.append({
            "idx": idx_w[c],
            "meta": meta[c],
            "xt": np.ascontiguousarray(xp[c].T),
            "w_in_t": w_in_t,
            "w2": w2,
            "bias_b": bias_b,
            "gamma_b": gamma_b,
            "beta_b": beta_b,
            "b_in_b": b_in_bc,
            "iota": iota,
            "ident": ident,
            "cnt": np.ascontiguousarray(
                counts_pc[c].reshape(1, TILES * BUCKETS)),
            "invb": invb[c],
            **({"ohp": oh_host[c]} if DBG_HOSTOH else {}),
        })
    return in_maps, l_bank, affine_trivial


def kernel(**inputs):
    in_maps, l_bank, affine_trivial = _preprocess(
        np.asarray(inputs["x"]), np.asarray(inputs["edge_src"]),
        np.asarray(inputs["edge_dst"]), np.asarray(inputs["W_in"]),
        np.asarray(inputs["b_in"]), np.asarray(inputs["Ws_self"]),
        np.asarray(inputs["Ws_neigh"]), np.asarray(inputs["biases"]),
        np.asarray(inputs["gammas"]), np.asarray(inputs["betas"]))

    key = (l_bank, affine_trivial, DBG_NQ, DBG_HOSTOH, DBG_SP, DBG_SCRATCH)
    if key not in _program_cache:
        _program_cache[key] = _build_program(l_bank, affine_trivial)
    nc = _program_cache[key]

    res = run_bass_kernel_spmd(nc, in_maps, list(range(NCORES)))
    out = np.concatenate(
        [res.results[c]["h_out"][:NODES_PER_CORE] for c in range(NCORES)],
        axis=0)
    return out.astype(np.float32)


# revision 33
# speedup vs baseline: 1.2306x; 1.0084x over previous
"""Trainium2 Bass kernel for DrBCEncoder-style GNN message passing.

Strategy (8 NeuronCores, SPMD, dst-sharded nodes):
  - Nodes dst-sharded: core c owns rows [c*12500, (c+1)*12500), padded to
    12544 = 98*128 rows (total padded node space 100352).
  - Activations in HBM per layer:
      h_full  [50176, 128] bf16 = 100352 packed 128-byte rows viewed as
              256B row-PAIRS (the dma_gather elem granularity). AllGather
              output, gather source.
      shard   [12544, 64] f32 local shard (residual source, exact).
  - Per 128-dst tile: edges bucketed per (tile, bank, src-parity); the
    2 banks split the pair-index space so indices fit int16 (25088 < 32768).
    Gather elem = 256B = the PAIR containing src; the correct half is
    selected statically per chunk via the matmul stationary slice
    feats[:, k, 64*parity : 64*parity+64] (each bucket is single-parity).
    Edges sorted by src within bucket for HBM row locality. Exact per-call
    counts via count registers (8-deep rotation); pads are trailing idx=-1
    (desc-gen skips them).
  - segment-sum as matmul: PSUM[64f, 128dst] += feats_k[128e, 64f].T @
    oh_k[128e, 128dst] with host-built one-hot (inv_deg folded) per tile.
  - Self+neigh projection fused: stat128 = [hT; nmT] (hT via PE transpose
    of the bf16 shard row tile, nmT copied from the agg PSUM), then one
    matmul z[128n, 64] = stat128.T @ [Ws'; Wn'].
  - LayerNorm in f32 on the free axis, relu + residual (f32 shard), then
    store f32 shard + bf16 packed shard, AllGather for the next layer.

Host-side work is index preprocessing only (edge sort/bucketing, degree
bincount, layout packing, weight transposes, bf16 casts).
"""
import sys

sys.path.insert(0, "/opt/trn_rl_repo")

import ml_dtypes
import numpy as np

import concourse.bass as bass
import concourse.bacc as bacc
import concourse.tile as tile
from concourse import mybir
from concourse.bass_utils import run_bass_kernel_spmd

NCORES = 8
N_NODES = 100000
NODES_PER_CORE = 12500
PAD_PER_CORE = 12544            # 98 * 128
N_PAD = NCORES * PAD_PER_CORE   # 100352
TILES = PAD_PER_CORE // 128     # 98
HALF = PAD_PER_CORE // 2        # 6272 rows per AllGather half
HALF_T = HALF // 128            # 49 tiles per half
PAIRS = N_PAD // 2              # 50176 256B pair-rows
BANKS = 2                       # bank b = all cores' half-b rows
BANK_PAIRS = NCORES * HALF // 2  # 25088 (< 32768 for int16 indices)
BUCKETS = 4                     # (bank << 1) | src_parity
HID = 64
ROW = 2 * HID                   # gather elem: 128 bf16 = 256B = 2 rows
IN_DIM = 8
N_LAYERS = 3
LN_EPS = 1e-5

F32 = mybir.dt.float32
BF16 = mybir.dt.bfloat16
F8 = mybir.dt.float8e4
I16 = mybir.dt.int16
AOT = mybir.AluOpType
ACT_F = mybir.ActivationFunctionType
BF = ml_dtypes.bfloat16
F8NP = mybir.dt.np(mybir.dt.float8e4)

_program_cache = {}

import os
DBG_NQ = int(os.environ.get("GNN_NQ", "4"))        # gather queues used
DBG_HOSTOH = os.environ.get("GNN_HOSTOH", "1") == "1"  # host-precomputed oh
DBG_SORT = os.environ.get("GNN_SORT", "1") == "1"  # sort buckets by src
DBG_SP = os.environ.get("GNN_SP", "0") == "1"      # force single_packet
DBG_SCRATCH = int(os.environ.get("GNN_SCRATCH", "65536"))


def _remap(v):
    return (v // NODES_PER_CORE) * PAD_PER_CORE + (v % NODES_PER_CORE)


def _build_program(l_bank, affine_trivial):
    """SPMD Bass program. l_bank: padded edges per (tile, bucket) slot.
    affine_trivial: gammas==1 and betas==0, skip the two affine ops."""
    cb = l_bank // 128          # chunks per bucket
    C = BUCKETS * cb            # chunks per tile
    lb16 = l_bank // 16

    nc = bacc.Bacc("TRN2", target_bir_lowering=False, debug=False,
                   num_devices=NCORES, num_swdge_queues=DBG_NQ,
                   dynamic_dma_scratch_size=DBG_SCRATCH)

    idx_in = nc.dram_tensor("idx", [TILES, 128, BUCKETS * lb16], I16,
                            kind="ExternalInput")
    meta_in = nc.dram_tensor("meta", [TILES, 128, 2 * C], F32,
                             kind="ExternalInput")
    xt_in = nc.dram_tensor("xt", [IN_DIM, PAD_PER_CORE], F32,
                           kind="ExternalInput")
    w_in_t = nc.dram_tensor("w_in_t", [IN_DIM, HID], F32, kind="ExternalInput")
    w2_in = nc.dram_tensor("w2", [N_LAYERS, 2 * HID, HID], BF16,
                           kind="ExternalInput")
    bias_b = nc.dram_tensor("bias_b", [N_LAYERS, 128, HID], F32,
                            kind="ExternalInput")
    gamma_b = nc.dram_tensor("gamma_b", [N_LAYERS, 128, HID], F32,
                             kind="ExternalInput")
    beta_b = nc.dram_tensor("beta_b", [N_LAYERS, 128, HID], F32,
                            kind="ExternalInput")
    b_in_b = nc.dram_tensor("b_in_b", [128, HID], F32, kind="ExternalInput")
    iota_in = nc.dram_tensor("iota", [128, 128], BF16, kind="ExternalInput")
    ident_in = nc.dram_tensor("ident", [128, 128], BF16, kind="ExternalInput")
    cnt_in = nc.dram_tensor("cnt", [1, TILES * BUCKETS], mybir.dt.int32,
                            kind="ExternalInput")
    oh_in = None
    if DBG_HOSTOH:
        oh_in = nc.dram_tensor("ohp", [TILES, 128, C * 128], F8,
                               kind="ExternalInput")
    invb_in = nc.dram_tensor("invb", [HID, TILES * 128], BF16,
                             kind="ExternalInput")
    h_out = nc.dram_tensor("h_out", [PAD_PER_CORE, HID], F32,
                           kind="ExternalOutput")

    with tile.TileContext(nc) as tc:
        with (
            tc.tile_pool(name="const", bufs=1) as cp,
            tc.tile_pool(name="io", bufs=6) as iop,
            tc.tile_pool(name="feats", bufs=6) as fp,
            tc.tile_pool(name="oh", bufs=(3 if DBG_HOSTOH else 8)) as ohp,
            tc.tile_pool(name="ln", bufs=3) as lnp,
            tc.tile_pool(name="hb", bufs=3) as hbp,
            tc.tile_pool(name="st", bufs=3) as stp,
            tc.tile_pool(name="ps_agg", bufs=2, space="PSUM") as ps_agg,
            tc.tile_pool(name="ps_tp", bufs=2, space="PSUM") as ps_tp,
            tc.tile_pool(name="ps_z", bufs=2, space="PSUM") as ps_z,
            tc.tile_pool(name="dram", bufs=1, space="DRAM") as dp,
        ):
            # ---- constants ----
            identb_t = cp.tile([128, 128], BF16, tag="identb")
            nc.sync.dma_start(identb_t[:], ident_in[:])
            iota_t = None
            if not DBG_HOSTOH:
                iota_t = cp.tile([128, 128], BF16, tag="iota")
                nc.sync.dma_start(iota_t[:], iota_in[:])
            cnt_sb = cp.tile([1, TILES * BUCKETS], mybir.dt.int32, tag="cnt")
            nc.sync.dma_start(cnt_sb[:], cnt_in[:])
            eps_t = cp.tile([128, 1], F32, tag="eps")
            nc.vector.memset(eps_t[:], LN_EPS)
            w_in_sb = cp.tile([IN_DIM, HID], F32, tag="w_in")
            nc.sync.dma_start(w_in_sb[:], w_in_t[:])
            b_in_sb = cp.tile([128, HID], F32, tag="b_in")
            nc.sync.dma_start(b_in_sb[:], b_in_b[:])
            invb_sb = cp.tile([HID, TILES * 128], BF16, tag="invb")
            nc.sync.dma_start(invb_sb[:], invb_in[:])
            w2_sb, bias_sb, gamma_sb, beta_sb = [], [], [], []
            for l in range(N_LAYERS):
                w1 = cp.tile([2 * HID, HID], BF16, tag=f"w2_{l}")
                nc.sync.dma_start(w1[:], w2_in[l])
                w2_sb.append(w1)
                b1 = cp.tile([128, HID], F32, tag=f"bias{l}")
                nc.sync.dma_start(b1[:], bias_b[l])
                bias_sb.append(b1)
                if not affine_trivial:
                    g1 = cp.tile([128, HID], F32, tag=f"gamma{l}")
                    nc.sync.dma_start(g1[:], gamma_b[l])
                    gamma_sb.append(g1)
                    be1 = cp.tile([128, HID], F32, tag=f"beta{l}")
                    nc.sync.dma_start(be1[:], beta_b[l])
                    beta_sb.append(be1)

            # ---- DRAM buffers ----
            # gather bank b = AllGather of all cores' half-b shard rows, so
            # the first AllGather can fire mid-layer (after tile HALF_T-1).
            h_bufs = [
                [dp.tile([BANK_PAIRS, ROW], BF16, tag=f"h_buf{i}_{b}",
                         name=f"h_buf{i}_{b}", addr_space="Shared")
                 for b in range(BANKS)]
                for i in range(N_LAYERS)
            ]
            pads = [
                dp.tile([PAD_PER_CORE, HID], BF16, tag=f"pad{i}",
                        name=f"pad{i}")
                for i in range(N_LAYERS)
            ]
            shards = [
                dp.tile([PAD_PER_CORE, HID], F32, tag=f"shard{i}",
                        name=f"shard{i}")
                for i in range(N_LAYERS)
            ]

            # zero the feats pool buffers once: gather skips trailing pad
            # slots (idx=-1) leaving stale bytes that must stay finite.
            for _ in range(6):
                fz = fp.tile([128, C, ROW], BF16, tag="feats")
                nc.vector.memset(fz[:], 0.0)

            # ---- phase 0: h0 = relu(x @ W_in.T + b_in) for own shard ----
            for t in range(TILES):
                xt_sb = iop.tile([IN_DIM, 128], F32, tag="xt")
                nc.sync.dma_start(xt_sb[:], xt_in[:, t * 128:(t + 1) * 128])
                h0_ps = ps_z.tile([128, HID], F32, tag="z")
                nc.tensor.matmul(h0_ps[:], xt_sb[:], w_in_sb[:],
                                 start=True, stop=True)
                h0_sb = lnp.tile([128, HID], F32, tag="hnew")
                nc.vector.scalar_tensor_tensor(
                    h0_sb[:], h0_ps[:], 0.0, b_in_sb[:], AOT.bypass, AOT.add)
                h0r_sb = lnp.tile([128, HID], F32, tag="hnew2")
                nc.scalar.activation(h0r_sb[:], h0_sb[:], ACT_F.Relu)
                hb = hbp.tile([128, HID], BF16, tag="hb")
                nc.scalar.copy(hb[:], h0r_sb[:])
                nc.sync.dma_start(shards[0][t * 128:(t + 1) * 128, :],
                                  h0r_sb[:])
                nc.sync.dma_start(pads[0][t * 128:(t + 1) * 128, :], hb[:])
                if t == HALF_T - 1:
                    nc.gpsimd.collective_compute(
                        "AllGather", AOT.bypass,
                        ins=[pads[0][0:HALF, :].opt()],
                        outs=[h_bufs[0][0].opt()],
                        replica_groups=[list(range(NCORES))])
            nc.gpsimd.collective_compute(
                "AllGather", AOT.bypass,
                ins=[pads[0][HALF:, :].opt()], outs=[h_bufs[0][1].opt()],
                replica_groups=[list(range(NCORES))])

            # ---- layers ----
            # depth-8 register rotation per bucket: the WAR dep between a
            # gather and the count reload for the same register otherwise
            # head-of-line-blocks the Pool sequencer and serializes queues.
            RDEPTH = 8
            cnt_regs = [[nc.gpsimd.alloc_register(f"cnt{b}_{r}")
                         for b in range(BUCKETS)] for r in range(RDEPTH)]
            for l in range(N_LAYERS):
                last = l == N_LAYERS - 1
                for t in range(TILES):
                    im_t = iop.tile([128, BUCKETS * lb16], I16, tag="idx")
                    nc.sync.dma_start(im_t[:], idx_in[t])
                    if not DBG_HOSTOH:
                        meta_t = iop.tile([128, 2 * C], F32, tag="meta")
                        nc.sync.dma_start(meta_t[:], meta_in[t])
                        meta = meta_t

                    feats = fp.tile([128, C, ROW], BF16, tag="feats")
                    for u in range(BUCKETS):
                        bank = u >> 1
                        g = t * BUCKETS + u
                        nreg = cnt_regs[t % RDEPTH][u]
                        nc.gpsimd.reg_load(nreg, cnt_sb[0:1, g:g + 1])
                        nc.gpsimd.dma_gather(
                            feats[:, u * cb:(u + 1) * cb, :],
                            h_bufs[l][bank][:],
                            im_t[:, u * lb16:(u + 1) * lb16],
                            l_bank, nreg, ROW,
                            single_packet=False,
                            queue_num=u % DBG_NQ)

                    agg = ps_agg.tile([HID, 128], F32, tag="agg")
                    if DBG_HOSTOH:
                        oh_t = ohp.tile([128, C * 128], F8, tag="oh")
                        nc.sync.dma_start(oh_t[:], oh_in[t])
                        for k in range(C):
                            par = (k // cb) & 1
                            nc.tensor.matmul(
                                agg[:],
                                feats[:, k, HID * par:HID * par + HID],
                                oh_t[:, k * 128:(k + 1) * 128],
                                start=(k == 0), stop=(k == C - 1))
                    else:
                        for k in range(C):
                            par = (k // cb) & 1
                            oh = ohp.tile([128, 128], BF16, tag="oh")
                            nc.vector.tensor_scalar(
                                oh[:], iota_t[:],
                                meta[:, k:k + 1], None, AOT.is_equal)
                            nc.tensor.matmul(
                                agg[:],
                                feats[:, k, HID * par:HID * par + HID],
                                oh[:],
                                start=(k == 0), stop=(k == C - 1))

                    # stat128 = [hT ; nmT] for the fused z matmul.
                    # nmT = agg * inv_deg[dst]; tensor_tensor is the 1-port
                    # DVE class (no SWDGE port convoy).
                    stat128 = stp.tile([128, 128], BF16, tag="stat")
                    nc.vector.tensor_tensor(
                        stat128[HID:128, :], agg[:],
                        invb_sb[:, t * 128:(t + 1) * 128], AOT.mult)

                    h_t = iop.tile([128, HID], F32, tag="h_t")
                    nc.scalar.dma_start(
                        h_t[:], shards[l][t * 128:(t + 1) * 128, :])
                    # ACT copy, NOT nc.vector: a DVE cast enters 2-port perf
                    # mode and locks GpSimd out of the shared SBUF port,
                    # stalling SWDGE descriptor generation for the gathers.
                    hbt = lnp.tile([128, HID], BF16, tag="hbt")
                    nc.scalar.copy(hbt[:], h_t[:])
                    tp_ps = ps_tp.tile([HID, 128], BF16, tag="tp")
                    nc.tensor.transpose(tp_ps[:], hbt[:], identb_t[:])
                    nc.scalar.copy(stat128[0:HID, :], tp_ps[:])

                    z_ps = ps_z.tile([128, HID], F32, tag="z")
                    nc.tensor.matmul(z_ps[:], stat128[:], w2_sb[l][:],
                                     start=True, stop=True)

                    # LayerNorm + affine + relu + residual
                    stats = lnp.tile([128, 2], F32, tag="stats")
                    zb = lnp.tile([128, HID], F32, tag="zb")
                    nc.vector.scalar_tensor_tensor(
                        zb[:], z_ps[:], 0.0, bias_sb[l][:],
                        AOT.bypass, AOT.add, accum_out=stats[:, 0:1])
                    zsq = lnp.tile([128, HID], F32, tag="zsq")
                    nc.scalar.activation(zsq[:], zb[:], ACT_F.Square,
                                         accum_out=stats[:, 1:2])
                    # tensor_scalar enters DVE 2-port perf mode and convoys
                    # with SWDGE — use scalar_tensor_tensor / tensor_tensor
                    # (1-port) and ACT scale+bias instead.
                    mstat = lnp.tile([128, 2], F32, tag="mstat")
                    nc.vector.scalar_tensor_tensor(
                        mstat[:], stats[:, 0:2], 1.0 / HID, stats[:, 0:2],
                        AOT.mult, AOT.bypass)
                    m2 = lnp.tile([128, 1], F32, tag="m2")
                    nc.vector.tensor_tensor(
                        m2[:], mstat[:, 0:1], mstat[:, 0:1], AOT.mult)
                    var = lnp.tile([128, 1], F32, tag="var")
                    nc.vector.tensor_tensor(
                        var[:], mstat[:, 1:2], m2[:], AOT.subtract)
                    std = lnp.tile([128, 1], F32, tag="std")
                    nc.scalar.activation(std[:], var[:], ACT_F.Sqrt,
                                         bias=eps_t[:])
                    rstd = lnp.tile([128, 1], F32, tag="rstd")
                    nc.vector.reciprocal(rstd[:], std[:])
                    mr = lnp.tile([128, 1], F32, tag="mr")
                    nc.vector.tensor_tensor(
                        mr[:], mstat[:, 0:1], rstd[:], AOT.mult)
                    nmr = lnp.tile([128, 1], F32, tag="nmr")
                    nc.vector.scalar_tensor_tensor(
                        nmr[:], mr[:], -1.0, mr[:], AOT.mult, AOT.bypass)
                    t2 = lnp.tile([128, HID], F32, tag="t2")
                    nc.scalar.activation(t2[:], zb[:], ACT_F.Identity,
                                         bias=nmr[:], scale=rstd[:])
                    t4 = t2
                    if not affine_trivial:
                        t3 = lnp.tile([128, HID], F32, tag="t3")
                        nc.vector.scalar_tensor_tensor(
                            t3[:], t2[:], 0.0, gamma_sb[l][:],
                            AOT.bypass, AOT.mult)
                        t4 = lnp.tile([128, HID], F32, tag="t4")
                        nc.vector.scalar_tensor_tensor(
                            t4[:], t3[:], 0.0, beta_sb[l][:],
                            AOT.bypass, AOT.add)
                    h_new = lnp.tile([128, HID], F32, tag="hnew")
                    nc.vector.scalar_tensor_tensor(
                        h_new[:], t4[:], 0.0, h_t[:], AOT.max, AOT.add)

                    if last:
                        nc.sync.dma_start(
                            h_out[t * 128:(t + 1) * 128, :], h_new[:])
                    else:
                        hb = hbp.tile([128, HID], BF16, tag="hb")
                        nc.scalar.copy(hb[:], h_new[:])
                        nc.sync.dma_start(
                            shards[l + 1][t * 128:(t + 1) * 128, :], h_new[:])
                        nc.sync.dma_start(
                            pads[l + 1][t * 128:(t + 1) * 128, :], hb[:])
                        if t == HALF_T - 1:
                            nc.gpsimd.collective_compute(
                                "AllGather", AOT.bypass,
                                ins=[pads[l + 1][0:HALF, :].opt()],
                                outs=[h_bufs[l + 1][0].opt()],
                                replica_groups=[list(range(NCORES))])
                if not last:
                    nc.gpsimd.collective_compute(
                        "AllGather", AOT.bypass,
                        ins=[pads[l + 1][HALF:, :].opt()],
                        outs=[h_bufs[l + 1][1].opt()],
                        replica_groups=[list(range(NCORES))])

    nc.compile()
    return nc


def _preprocess(x, edge_src, edge_dst, W_in, b_in, Ws_self, Ws_neigh,
                biases, gammas, betas):
    """Pure index/layout preprocessing on the host."""
    src = edge_src.astype(np.int64)
    dst = edge_dst.astype(np.int64)
    rsrc = _remap(src)
    rdst = _remap(dst)

    tile_g = rdst // 128              # global tile id in padded space, 0..783
    dst_loc = (rdst % 128).astype(np.int64)
    # bank b holds the AllGather of all cores' half-b rows:
    # h_bufs[l][b][c*HALF/2 + jb/2] = 256B pair of core c rows (jb, jb+1)
    src_core = rsrc // PAD_PER_CORE
    src_j = rsrc % PAD_PER_CORE
    bank = (src_j >= HALF).astype(np.int64)
    jb = src_j - bank * HALF
    parity = (rsrc & 1).astype(np.int64)
    idx_loc = (src_core * (HALF // 2) + (jb >> 1)).astype(np.int16)

    deg = np.bincount(dst, minlength=N_NODES)
    invdeg = np.where(deg > 0, 1.0 / np.maximum(deg, 1), 0.0).astype(np.float32)
    inv_e = invdeg[dst]

    n_groups = NCORES * TILES * BUCKETS
    bucket = (bank << 1) | parity
    key = tile_g * BUCKETS + bucket
    if DBG_SORT:
        order = np.lexsort((rsrc, key))
    else:
        order = np.argsort(key, kind="stable")
    key_s = key[order]
    counts = np.bincount(key_s, minlength=n_groups)
    l_bank = max(256, int(np.ceil(counts.max() / 128)) * 128)
    cb = l_bank // 128
    C = BUCKETS * cb
    lb16 = l_bank // 16

    starts = np.zeros(n_groups, dtype=np.int64)
    starts[1:] = np.cumsum(counts)[:-1]
    rank = np.arange(len(src)) - starts[key_s]
    pos = key_s * l_bank + rank       # global padded position

    total = n_groups * l_bank
    idx_full = np.full(total, -1, dtype=np.int16)  # pad: skipped by ucode
    idx_full[pos] = idx_loc[order]
    dstl_full = np.full(total, -1.0, dtype=np.float32)
    dstl_full[pos] = dst_loc[order].astype(np.float32)
    inv_full = np.zeros(total, dtype=np.float32)
    inv_full[pos] = inv_e[order]

    # idx: [784, BUCKETS, l_bank] -> wrap16 -> replicate to 128 partitions
    idx_w = idx_full.reshape(NCORES * TILES, BUCKETS, lb16, 16)
    idx_w = idx_w.transpose(0, 1, 3, 2)                # [784, U, 16, lb16]
    idx_w = np.broadcast_to(idx_w[:, :, None, :, :],
                            (NCORES * TILES, BUCKETS, 8, 16, lb16))
    idx_w = idx_w.transpose(0, 2, 3, 1, 4).reshape(
        NCORES, TILES, 128, BUCKETS * lb16)

    # meta: positions within a tile wrap mod 128 across all chunks
    dstl_w = dstl_full.reshape(NCORES * TILES, C, 128).transpose(0, 2, 1)
    inv_w = inv_full.reshape(NCORES * TILES, C, 128).transpose(0, 2, 1)
    meta = np.concatenate([dstl_w, inv_w], axis=2).reshape(
        NCORES, TILES, 128, 2 * C).astype(np.float32)
    idx_w = np.ascontiguousarray(idx_w)
    meta = np.ascontiguousarray(meta)

    oh_host = None
    if DBG_HOSTOH:
        # 0/1 one-hot tiles in fp8 (exact), edge-major partitions; inv_deg
        # is applied on-device per dst column (invb) instead of per edge.
        bucket_e = key_s % BUCKETS
        rank_e = rank  # within (tile, bucket) group, aligned with `order`
        tile_e = key_s // BUCKETS
        k_e = bucket_e * cb + rank_e // 128      # chunk within tile
        e_loc = rank_e % 128                      # partition within chunk
        flat = ((tile_e * 128 + e_loc) * C + k_e) * 128 + dst_loc[order]
        oh_host = np.zeros(NCORES * TILES * 128 * C * 128, dtype=np.uint8)
        one_f8 = np.ones((), dtype=F8NP).view(np.uint8)
        oh_host[flat] = one_f8
        oh_host = oh_host.view(F8NP).reshape(NCORES, TILES, 128, C * 128)

    # per-dst inv_deg, broadcast to 64 partitions: [NCORES, 64, 12544]
    invp = np.zeros(N_PAD, dtype=np.float32)
    invp[_remap(np.arange(N_NODES))] = invdeg
    invb = np.ascontiguousarray(np.broadcast_to(
        invp.reshape(NCORES, 1, PAD_PER_CORE),
        (NCORES, HID, PAD_PER_CORE)).astype(BF))

    # xT per core
    xp = np.zeros((N_PAD, IN_DIM), dtype=np.float32)
    xp[_remap(np.arange(N_NODES))] = x
    xp = xp.reshape(NCORES, PAD_PER_CORE, IN_DIM)

    w_in_t = np.ascontiguousarray(W_in.T.astype(np.float32))
    ws_t = Ws_self.transpose(0, 2, 1).astype(np.float32)
    wn_t = Ws_neigh.transpose(0, 2, 1).astype(np.float32)
    w2 = np.ascontiguousarray(
        np.concatenate([ws_t, wn_t], axis=1)).astype(BF)  # [L, 128, 64]
    bias_b = np.ascontiguousarray(
        np.broadcast_to(biases[:, None, :],
                        (N_LAYERS, 128, HID)).astype(np.float32))
    gamma_b = np.ascontiguousarray(
        np.broadcast_to(gammas[:, None, :],
                        (N_LAYERS, 128, HID)).astype(np.float32))
    beta_b = np.ascontiguousarray(
        np.broadcast_to(betas[:, None, :],
                        (N_LAYERS, 128, HID)).astype(np.float32))
    b_in_bc = np.ascontiguousarray(
        np.broadcast_to(b_in[None, :], (128, HID)).astype(np.float32))
    iota = np.tile(np.arange(128, dtype=np.float32), (128, 1)).astype(BF)
    ident = np.eye(128, dtype=np.float32).astype(BF)

    affine_trivial = bool(np.all(gammas == 1.0) and np.all(betas == 0.0))

    counts_pc = counts.reshape(NCORES, TILES, BUCKETS).astype(np.int32)

    in_maps = []
    for c in range(NCORES):
        in_maps.append({
            "idx": idx_w[c],
            "meta": meta[c],
            "xt": np.ascontiguousarray(xp[c].T),
            "w_in_t": w_in_t,
            "w2": w2,
            "bias_b": bias_b,
            "gamma_b": gamma_b,
            "beta_b": beta_b,
            "b_in_b": b_in_bc,
            "iota": iota,
            "ident": ident,
            "cnt": np.ascontiguousarray(
                counts_pc[c].reshape(1, TILES * BUCKETS)),
            "invb": invb[c],
            **({"ohp": oh_host[c]} if DBG_HOSTOH else {}),
        })
    return in_maps, l_bank, affine_trivial


def kernel(**inputs):
    in_maps, l_bank, affine_trivial = _preprocess(
        np.asarray(inputs["x"]), np.asarray(inputs["edge_src"]),
        np.asarray(inputs["edge_dst"]), np.asarray(inputs["W_in"]),
        np.asarray(inputs["b_in"]), np.asarray(inputs["Ws_self"]),
        np.asarray(inputs["Ws_neigh"]), np.asarray(inputs["biases"]),
        np.asarray(inputs["gammas"]), np.asarray(inputs["betas"]))

    key = (l_bank, affine_trivial, DBG_NQ, DBG_HOSTOH, DBG_SP, DBG_SCRATCH)
    if key not in _program_cache:
        _program_cache[key] = _build_program(l_bank, affine_trivial)
    nc = _program_cache[key]

    res = run_bass_kernel_spmd(nc, in_maps, list(range(NCORES)))
    out = np.concatenate(
        [res.results[c]["h_out"][:NODES_PER_CORE] for c in range(NCORES)],
        axis=0)
    return out.astype(np.float32)
